# revision 1
# baseline (speedup 1.0000x reference)
"""Trainium2 Bass kernel for nn_AggFeatureModel (segment_reduce).

Computes, per batch row b (B=4096, T=2048):
  - seq_len, sum/mean/std of amount over the full T axis
  - per-category (mcc: C=100, tr_type: C=50) count/mean/std of amount
  - distinct-category counts
Output: [B, 456] = [sl, s, mean, std, mcc_cnt(100), mcc_mean(100),
  mcc_std(100), tr_cnt(50), tr_mean(50), tr_std(50), dist_mcc, dist_tr]

Sharding: pure data parallel, B split across 8 NeuronCores (512 rows each).
"""

import sys

sys.path.insert(0, "/opt/trn_rl_repo")

from contextlib import ExitStack

import numpy as np

import concourse.bass as bass
import concourse.tile as tile
from concourse import bacc, mybir
from concourse.bass_utils import run_bass_kernel_spmd

B, T = 4096, 2048
NCORES = 8
RPC = B // NCORES  # rows per core
C_MCC, C_TR = 100, 50
EPS = 1e-9
OUT_COLS = 456
PT = 128  # partition tile (rows per SBUF tile)
NT = RPC // PT  # row tiles per core

F32 = mybir.dt.float32
BF16 = mybir.dt.bfloat16
I32 = mybir.dt.int32
AX = mybir.AxisListType.X
OP = mybir.AluOpType
AF = mybir.ActivationFunctionType


def _cat_stats_postproc(nc, pool, cnt, s, ss, out_tile, col0, C):
    """Given per-category cnt/s/ss [128, C] f32, write cnt/mean/std into
    out_tile columns [col0:col0+3C] and return distinct count [128,1]."""
    tmp = pool.tile([PT, C], F32, tag=f"pp_tmp_{C}")
    rec = pool.tile([PT, C], F32, tag=f"pp_rec_{C}")
    # cnt goes out directly
    nc.vector.tensor_copy(out_tile[:, col0 : col0 + C], cnt[:])
    # rec = 1/(cnt + EPS)
    nc.vector.tensor_scalar(tmp[:], cnt[:], EPS, None, OP.add)
    nc.vector.reciprocal(rec[:], tmp[:])
    # mean = s * rec
    mean = out_tile[:, col0 + C : col0 + 2 * C]
    nc.vector.tensor_tensor(mean, s[:], rec[:], OP.mult)
    # var_num = clip(ss - s*mean, 0)
    nc.vector.tensor_tensor(tmp[:], s[:], mean, OP.mult)
    nc.vector.tensor_tensor(tmp[:], ss[:], tmp[:], OP.subtract)
    nc.vector.tensor_scalar(tmp[:], tmp[:], 0.0, None, OP.max)
    # denom = clip(cnt-1, 0) + EPS ; rec = 1/denom
    nc.vector.tensor_scalar(rec[:], cnt[:], 1.0, 0.0, OP.subtract, OP.max)
    nc.vector.tensor_scalar(rec[:], rec[:], EPS, None, OP.add)
    nc.vector.reciprocal(rec[:], rec[:])
    nc.vector.tensor_tensor(tmp[:], tmp[:], rec[:], OP.mult)
    nc.scalar.sqrt(out_tile[:, col0 + 2 * C : col0 + 3 * C], tmp[:])
    # distinct = sum(cnt > 0)
    dist = pool.tile([PT, 1], F32, tag=f"pp_dist_{C}")
    nc.vector.tensor_scalar(tmp[:], cnt[:], 0.0, None, OP.is_gt)
    nc.vector.reduce_sum(dist[:], tmp[:], axis=AX)
    return dist


def _build_body(ctx, tc):
    nc = tc.nc
    amount_d = nc.dram_tensor("amount", [RPC, T], F32, kind="ExternalInput")
    mcc_d = nc.dram_tensor("mcc", [RPC, T], I32, kind="ExternalInput")
    tr_d = nc.dram_tensor("tr_type", [RPC, T], I32, kind="ExternalInput")
    seq_d = nc.dram_tensor("seq_lens", [RPC, 1], I32, kind="ExternalInput")
    out_d = nc.dram_tensor("out", [RPC, OUT_COLS], F32, kind="ExternalOutput")

    io_pool = ctx.enter_context(tc.tile_pool(name="io", bufs=2))
    work = ctx.enter_context(tc.tile_pool(name="work", bufs=2))
    acc_pool = ctx.enter_context(tc.tile_pool(name="acc", bufs=2))

    for it in range(NT):
        if it > 0:
            tc.strict_bb_all_engine_barrier()
        r0 = it * PT
        rows = slice(r0, r0 + PT)

        a = io_pool.tile([PT, T], F32, tag="a")
        nc.sync.dma_start(a[:], amount_d[rows, :])
        mcc_i = io_pool.tile([PT, T], I32, tag="mcc_i")
        nc.sync.dma_start(mcc_i[:], mcc_d[rows, :])
        tr_i = io_pool.tile([PT, T], I32, tag="tr_i")
        nc.sync.dma_start(tr_i[:], tr_d[rows, :])
        seq_i = io_pool.tile([PT, 1], I32, tag="seq_i")
        nc.sync.dma_start(seq_i[:], seq_d[rows, :])

        mcc_f = work.tile([PT, T], F32, tag="mcc_f")
        nc.vector.tensor_copy(mcc_f[:], mcc_i[:])
        tr_f = work.tile([PT, T], F32, tag="tr_f")
        nc.vector.tensor_copy(tr_f[:], tr_i[:])
        seq_f = work.tile([PT, 1], F32, tag="seq_f")
        nc.vector.tensor_copy(seq_f[:], seq_i[:])

        # bf16 working copies for the category loop
        mcc_bf = work.tile([PT, T], BF16, tag="mcc_bf")
        nc.vector.tensor_copy(mcc_bf[:], mcc_f[:])
        tr_bf = work.tile([PT, T], BF16, tag="tr_bf")
        nc.vector.tensor_copy(tr_bf[:], tr_f[:])
        a_bf = work.tile([PT, T], BF16, tag="a_bf")
        nc.vector.tensor_copy(a_bf[:], a[:])

        # a2 = a*a (bf16 for the loop), row sum of squares on scalar engine
        a2_bf = work.tile([PT, T], BF16, tag="a2_bf")
        ss_row = work.tile([PT, 1], F32, tag="ss_row")
        nc.scalar.activation(a2_bf[:], a[:], AF.Square, accum_out=ss_row[:])
        s_row = work.tile([PT, 1], F32, tag="s_row")
        nc.vector.reduce_sum(s_row[:], a[:], axis=AX)

        # Per-category accumulators
        cnt_m = acc_pool.tile([PT, C_MCC], F32, tag="cnt_m")
        s_m = acc_pool.tile([PT, C_MCC], F32, tag="s_m")
        ss_m = acc_pool.tile([PT, C_MCC], F32, tag="ss_m")
        cnt_t = acc_pool.tile([PT, C_TR], F32, tag="cnt_t")
        s_t = acc_pool.tile([PT, C_TR], F32, tag="s_t")
        ss_t = acc_pool.tile([PT, C_TR], F32, tag="ss_t")
        for t_ in (cnt_m, s_m, ss_m, cnt_t, s_t, ss_t):
            nc.vector.memset(t_[:, 0:1], 0.0)

        scr_g = work.tile([PT, T], BF16, tag="scr_g")
        scr_v = work.tile([PT, T], BF16, tag="scr_v")
        scr_v2 = work.tile([PT, T], BF16, tag="scr_v2")

        for cat_f, C, cnt, s, ss in (
            (mcc_bf, C_MCC, cnt_m, s_m, ss_m),
            (tr_bf, C_TR, cnt_t, s_t, ss_t),
        ):
            for c in range(1, C):
                fc = float(c)
                # count on gpsimd (independent engine), sums on vector
                nc.vector.tensor_scalar(
                    scr_g[:], cat_f[:], fc, 0.0, OP.is_equal, OP.add,
                    accum_out=cnt[:, c : c + 1],
                )
                nc.vector.scalar_tensor_tensor(
                    scr_v[:], cat_f[:], fc, a_bf[:], OP.is_equal, OP.mult,
                    accum_out=s[:, c : c + 1],
                )
                nc.vector.scalar_tensor_tensor(
                    scr_v2[:], cat_f[:], fc, a2_bf[:], OP.is_equal, OP.mult,
                    accum_out=ss[:, c : c + 1],
                )

        out_tile = acc_pool.tile([PT, OUT_COLS], F32, tag="out_tile")
        # col 0: seq_lens
        nc.vector.tensor_copy(out_tile[:, 0:1], seq_f[:])
        # col 1: s_row
        nc.vector.tensor_copy(out_tile[:, 1:2], s_row[:])
        # col 2: mean = s/(sl+EPS); col 3: std
        tmp1 = work.tile([PT, 1], F32, tag="tmp1")
        rec1 = work.tile([PT, 1], F32, tag="rec1")
        nc.vector.tensor_scalar(tmp1[:], seq_f[:], EPS, None, OP.add)
        nc.vector.reciprocal(rec1[:], tmp1[:])
        mean_row = out_tile[:, 2:3]
        nc.vector.tensor_tensor(mean_row, s_row[:], rec1[:], OP.mult)
        nc.vector.tensor_tensor(tmp1[:], s_row[:], mean_row, OP.mult)
        nc.vector.tensor_tensor(tmp1[:], ss_row[:], tmp1[:], OP.subtract)
        nc.vector.tensor_scalar(tmp1[:], tmp1[:], 0.0, None, OP.max)
        nc.vector.tensor_scalar(rec1[:], seq_f[:], 1.0, 0.0, OP.subtract, OP.max)
        nc.vector.tensor_scalar(rec1[:], rec1[:], EPS, None, OP.add)
        nc.vector.reciprocal(rec1[:], rec1[:])
        nc.vector.tensor_tensor(tmp1[:], tmp1[:], rec1[:], OP.mult)
        nc.scalar.sqrt(out_tile[:, 3:4], tmp1[:])

        dist_m = _cat_stats_postproc(nc, work, cnt_m, s_m, ss_m, out_tile, 4, C_MCC)
        dist_t = _cat_stats_postproc(
            nc, work, cnt_t, s_t, ss_t, out_tile, 4 + 3 * C_MCC, C_TR
        )
        nc.vector.tensor_copy(out_tile[:, 454:455], dist_m[:])
        nc.vector.tensor_copy(out_tile[:, 455:456], dist_t[:])

        nc.sync.dma_start(out_d[rows, :], out_tile[:])


_CACHED_NC = None


def _get_nc():
    global _CACHED_NC
    if _CACHED_NC is None:
        nc = bacc.Bacc(
            "TRN2",
            target_bir_lowering=False,
            debug=False,
            num_devices=NCORES,
        )
        with ExitStack() as ctx:
            tc = ctx.enter_context(tile.TileContext(nc))
            _build_body(ctx, tc)
        nc.finalize()
        _CACHED_NC = nc
    return _CACHED_NC


def kernel(amount, mcc, tr_type, seq_lens, trace=False, **trace_kwargs):
    nc = _get_nc()
    in_maps = []
    for i in range(NCORES):
        rows = slice(i * RPC, (i + 1) * RPC)
        in_maps.append(
            {
                "amount": np.ascontiguousarray(amount[rows], dtype=np.float32),
                "mcc": np.ascontiguousarray(mcc[rows], dtype=np.int32),
                "tr_type": np.ascontiguousarray(tr_type[rows], dtype=np.int32),
                "seq_lens": np.ascontiguousarray(
                    seq_lens[rows].reshape(RPC, 1), dtype=np.int32
                ),
            }
        )
    res = run_bass_kernel_spmd(
        nc, in_maps, list(range(NCORES)), trace=trace, **trace_kwargs
    )
    out = np.concatenate([r["out"] for r in res.results], axis=0)
    if trace:
        kernel.last_result = res
    return out

